# revision 30
# baseline (speedup 1.0000x reference)
"""Distributed attention layer kernel for 8 TRN2 NeuronCores.

Reference computation (f32):
    Q = q @ W_q; K = k @ W_k; V = v @ W_v
    out = softmax((Q @ K^T)/sqrt(d_k)) @ V

Sharding: rows of q/k/v are split 8 ways (sequence parallel). Each core
projects its own shards, the K^T/V projections are all-gathered (fp16),
and each core computes its 512-row slice of the attention output.

v5 restructure (from the traces of the v1-v4 kernels):
 - K path is minimal-latency: per-ct weight loads, it-major transposes
   (no mid-burst stalls), ct-outer two-pass projection (4 PSUM
   banks/pass), one half-tensor bounce per pass (DMA trigger cost on
   the Activation sequencer is ~0.7us each — fewer, bigger bounces),
   for the earliest possible K all-gather trigger. The gather end is
   gated by (launch skew + last core's K path).
 - V path runs before Q path; its bounce + gather are issued on the
   sync queue after the whole K^T prefetch (an active collective
   starves shared-DRAM reads), so the V transfer overlaps S row tiles
   1-3 and completes just before PV needs it.
 - S phase is it-outer (softmax + P^T of row tile it overlap S of
   it+1, removing the ~22us S->PV bubble). Row tile 0 is
   rr-middle/dtt-inner, streaming the gathered chunks at 1MB
   granularity; row tiles 1-3 run 4-chunk groups with one explicit
   LDWEIGHTS per stationary Q^T tile.
 - PV pairs each stationary P^T tile's two eh matmuls behind one
   explicit LDWEIGHTS.
 - Engine/queue split in S+PV: PSUM copies alternate Activation/DVE,
   reductions on DVE, exp on Activation, P^T half-transposes on the
   sync queue, V-chunk loads + output stores on the Activation queue.
 - Scores staging stays f32 (raw scores reach ~2.6e5, beyond fp16 max)
   but in a 2-buffer rotation: s_sb[it] is dead once exp(it) has run.
 - NOTE: bare ldweights=False WITHOUT a standalone InstLdweights
   partner is broken on HW (walrus ignores it for fp16 and emits
   garbage for f32r); only the explicit pairing below is safe.

Precision: projections run in f32r (full rate for free-dim >= 256) with
f32 PSUM accumulation; attention matmuls are fp16 with f32 accumulation;
softmax is f32 ACT exp with per-row max bias. Measured end-to-end error
vs the f32 reference: ~8e-3 (gate 2e-2).
"""

import os
import sys

for _p in ("/opt/pypackages", "/opt/trn_rl_repo"):
    if _p not in sys.path:
        sys.path.insert(0, _p)

import numpy as np

N_Q, N_KV, DIM = 4096, 4096, 1024  # D_K = D_V = DIM (square weights)
CORES = 8

P = 128

# Emit explicit InstLdweights + non-self-loading matmuls (fp16 only)
# where one stationary tile feeds several consecutive matmuls. The
# fused LDWEIGHTS+MATMUL pair measures 263ns for a 512-row fp16 matmul
# vs the 213ns matmul floor; explicit pairing recovers most of that 19%
# PE tax in S (4x reuse) and PV (2x reuse). (Setting ldweights=False
# WITHOUT a standalone InstLdweights partner is broken: walrus ignores
# it for fp16 and emits garbage for f32r.)
EXPLICIT_LDW = False


def build_attention(nq=N_Q, dim=DIM, cores=CORES):
    """Build the per-core Bass graph (SPMD; identical on all cores)."""
    import concourse.bass as bass
    import concourse.mybir as mybir
    from concourse import bacc
    from concourse.masks import make_identity
    from concourse.tile import TileContext

    dt = mybir.dt
    f32, f32r, f16 = dt.float32, dt.float32r, dt.float16

    sh = nq // cores          # rows per core (512)
    n_ct = dim // P           # contraction tiles for projections (8)
    n_dt = dim // P           # d tiles (8)
    n_it = sh // P            # query-row tiles per core (4)
    n_jt = nq // P            # total kv j tiles (32)
    JG = 4                    # j-tiles per PV V-chunk
    n_jg = n_jt // JG         # V chunk count (8)
    EH = 512
    n_eh = dim // EH          # 512-wide output column halves (2)
    scale = 1.0 / float(np.sqrt(dim))

    nc = bacc.Bacc(num_devices=cores)

    # --- external I/O (per core: row shards of q/k/v, full weights) ---
    q_ext = nc.declare_dram_parameter("q", [sh, dim], f32, isOutput=False)
    k_ext = nc.declare_dram_parameter("k", [sh, dim], f32, isOutput=False)
    v_ext = nc.declare_dram_parameter("v", [sh, dim], f32, isOutput=False)
    wq_ext = nc.declare_dram_parameter("W_q", [dim, dim], f32r, isOutput=False)
    wk_ext = nc.declare_dram_parameter("W_k", [dim, dim], f32r, isOutput=False)
    wv_ext = nc.declare_dram_parameter("W_v", [dim, dim], f32r, isOutput=False)
    out_ext = nc.declare_dram_parameter("out", [sh, dim], f32, isOutput=True)

    # --- internal DRAM for collectives ---
    bounce_k = nc.dram_tensor("bounce_k", [dim, sh], f16)
    bounce_v = nc.dram_tensor("bounce_v", [sh, dim], f16)
    gath_k = nc.dram_tensor("gath_k", [cores * dim, sh], f16, addr_space="Shared")
    gath_v = nc.dram_tensor("gath_v", [cores * sh, dim], f16, addr_space="Shared")

    rg = [list(range(cores))]

    def group_ldw(stationary):
        """Load a stationary fp16 tile once for the following group of
        matmuls. Returns a function wrapping nc.tensor.matmul that marks
        the matmul non-self-loading (pairs with the explicit load)."""
        if EXPLICIT_LDW:
            nc.tensor.ldweights(stationary)

            def mm(*a, **kw):
                bi = nc.tensor.matmul(*a, **kw)
                bi.ins.ldweights = False
                return bi
            return mm
        return nc.tensor.matmul

    with TileContext(nc) as tc:
        with (
            tc.tile_pool(name="const", bufs=1) as constp,
            tc.tile_pool(name="qt", bufs=1) as qtp,
            tc.tile_pool(name="stats", bufs=1) as statp,
        ):
            # NOTE: make_identity/PE-transpose on float32r crashes walrus
            # codegen; transposes run in plain f32 and the psum result is
            # copy-cast (bit-identical) into float32r SBUF tiles.
            ident_f = constp.tile([P, P], f32, tag="idf", name="idf")
            make_identity(nc, ident_f)

            qthi = qtp.tile([P, n_dt, sh], f16, tag="qthi", name="qthi")
            v_loc = qtp.tile([P, sh // P, dim], f16, tag="v_loc", name="v_loc")

            with (
                tc.tile_pool(name="w", bufs=1) as wpool,
                tc.tile_pool(name="iost", bufs=6) as iost,
                tc.tile_pool(name="tin", bufs=2) as tpool,
                tc.tile_pool(name="kvout", bufs=1) as kvout,
                tc.tile_pool(name="tpsum", bufs=4, space="PSUM") as tpsum,
                tc.tile_pool(name="ppsum", bufs=4, space="PSUM") as ppsum,
            ):
                # Bulk loads stream in K-path-first order on the sync (SP)
                # HWDGE queue; weights load per-ct so the ct-outer
                # projections can start before the full 4MB arrives. The
                # Activation HWDGE queue is reserved for latency-critical
                # transfers (bounce tiles, P^T XBAR transposes, outputs).
                def load_input(x_ext):
                    stgs = []
                    xsrc = x_ext.rearrange("(it p) c -> p it c", p=P)
                    for it in range(sh // P):
                        stg = iost.tile([P, dim], f32, tag="iostg", name="iostg")
                        nc.sync.dma_start(stg[:], xsrc[:, it])
                        stgs.append(stg)
                    return stgs

                wk = wpool.tile([P, n_ct, dim], f32r, tag="wk", name="wk")
                wv = wpool.tile([P, n_ct, dim], f32r, tag="wv", name="wv")
                wq = wpool.tile([P, n_ct, dim], f32r, tag="wq", name="wq")
                wk_src = wk_ext.rearrange("(ct p) d -> p ct d", p=P)
                wq_src = wq_ext.rearrange("(ct p) d -> p ct d", p=P)
                wv_src = wv_ext.rearrange("(ct p) d -> p ct d", p=P)

                k_stg = load_input(k_ext)
                for ct in range(n_ct):
                    nc.sync.dma_start(wk[:, ct], wk_src[:, ct])
                v_stg = load_input(v_ext)
                for ct in range(n_ct):
                    nc.sync.dma_start(wv[:, ct], wv_src[:, ct])
                q_stg = load_input(q_ext)
                for ct in range(n_ct):
                    nc.sync.dma_start(wq[:, ct], wq_src[:, ct])

                def transpose_input(stgs, tag):
                    """Transpose a staged [sh, dim] f32 input on the PE into a
                    [c_in=128, ct, row] f32r SBUF tile (copy-cast from psum).
                    it-major: each staged row tile is consumed in one 8-long
                    back-to-back burst as it lands, so the PE never stalls
                    (stalls reset the pstate ramp)."""
                    xt = tpool.tile([P, n_ct, sh], f32r, tag=tag, name=tag)
                    for it, stg in enumerate(stgs):
                        dst = slice(it * P, (it + 1) * P)
                        for ct in range(n_ct):
                            ps = tpsum.tile([P, P], f32, tag="tps", name="tps")
                            nc.tensor.transpose(
                                ps[:], stg[:, ct * P:(ct + 1) * P], ident_f
                            )
                            nc.vector.tensor_copy(xt[:, ct, dst], ps[:])
                    return xt

                def copy_eng(i):
                    return nc.scalar.copy if i % 2 == 0 else nc.vector.tensor_copy

                # ---- K path first: project K^T ct-outer in two 4-bank
                # passes, bounce each dtt tile as its copy lands, then
                # all-gather. The gather end is gated by the LAST core's
                # trigger (launch skew), so every us saved here moves the
                # whole S phase earlier. ----
                kt = transpose_input(k_stg, "xt")
                kt_loc = kvout.tile([P, n_dt, sh], f16, tag="kt_loc", name="kt_loc")
                bk = bounce_k.rearrange("(dtt p) jj -> p dtt jj", p=P)

                def project_dt(w_t, x_t, out_cb):
                    """out[dtt] = (W^T X^T)[dtt] for all 8 dtt column tiles,
                    two ct-outer passes of 4 PSUM banks each. out_cb(dtt, ps)
                    consumes the finished [P, sh] psum tile."""
                    for g in range(2):
                        dts = range(4 * g, 4 * g + 4)
                        pss = {
                            dtt: ppsum.tile([P, sh], f32, tag="pps", name="pps")
                            for dtt in dts
                        }
                        for ct in range(n_ct):
                            for dtt in dts:
                                dsl = slice(dtt * P, (dtt + 1) * P)
                                nc.tensor.matmul(
                                    pss[dtt][:], w_t[:, ct, dsl], x_t[:, ct],
                                    start=(ct == 0), stop=(ct == n_ct - 1),
                                )
                        for i, dtt in enumerate(dts):
                            out_cb(i, dtt, pss[dtt])

                def k_out(i, dtt, ps):
                    copy_eng(i)(kt_loc[:, dtt], ps[:])
                    if dtt in (3, 7):
                        # one half-tensor bounce per 4-bank pass: a single
                        # ~0.7us DMA trigger instead of four (the Activation
                        # sequencer cost per trigger, not the bytes, is what
                        # delays the gather trigger)
                        hs = slice(dtt - 3, dtt + 1)
                        nc.scalar.dma_start(bk[:, hs], kt_loc[:, hs])

                project_dt(wk, kt, k_out)
                nc.gpsimd.collective_compute(
                    "AllGather", mybir.AluOpType.bypass, replica_groups=rg,
                    ins=[bounce_k.ap().opt()], outs=[gath_k.ap().opt()],
                )

                # ---- V path second: project the V shard (jjt-pair passes).
                # No bounce here: the V all-gather's DRAM traffic would
                # starve the K^T chunk prefetch right after the K gather, so
                # the bounce DMA is issued on the sync queue mid-prefetch
                # (see the attention phase below). ----
                vt = transpose_input(v_stg, "xt")
                for g in range(2):
                    jjts = range(2 * g, 2 * g + 2)
                    pss = {
                        (jjt, eh): ppsum.tile([P, EH], f32, tag="pps", name="pps")
                        for jjt in jjts for eh in range(n_eh)
                    }
                    for ct in range(n_ct):
                        for jjt in jjts:
                            jsl = slice(jjt * P, (jjt + 1) * P)
                            for eh in range(n_eh):
                                esl = slice(eh * EH, (eh + 1) * EH)
                                nc.tensor.matmul(
                                    pss[(jjt, eh)][:], vt[:, ct, jsl],
                                    wv[:, ct, esl],
                                    start=(ct == 0), stop=(ct == n_ct - 1),
                                )
                    for i, (jjt, eh) in enumerate(pss):
                        esl = slice(eh * EH, (eh + 1) * EH)
                        copy_eng(i)(v_loc[:, jjt, esl], pss[(jjt, eh)][:])

                # ---- Q path last (local only; needed first at S start) ----
                qt = transpose_input(q_stg, "xt")

                def q_out(i, dtt, ps):
                    copy_eng(i)(qthi[:, dtt], ps[:])

                project_dt(wq, qt, q_out)

            # ================= attention phase =================
            m_t = [statp.tile([P, 1], f32, tag=f"m{it}", name=f"m{it}") for it in range(n_it)]
            tmpmax = statp.tile([P, 1], f32, tag="tmpmax", name="tmpmax")
            bias_t = [statp.tile([P, 1], f32, tag=f"b{it}", name=f"b{it}") for it in range(n_it)]
            ell_t = [statp.tile([P, 1], f32, tag=f"l{it}", name=f"l{it}") for it in range(n_it)]
            rl_t = [statp.tile([P, 1], f32, tag=f"r{it}", name=f"r{it}") for it in range(n_it)]

            gk = gath_k.rearrange("(r dtt p) jj -> r p dtt jj", r=cores, p=P)
            gv = gath_v.rearrange("(jg jj p) e -> jg p jj e", jj=JG, p=P)

            with (
                tc.tile_pool(name="kall", bufs=1) as kallp,
                tc.tile_pool(name="srow", bufs=2) as srow,
                tc.tile_pool(name="prow", bufs=2) as prow,
                tc.tile_pool(name="ptp", bufs=1) as ptp,
                tc.tile_pool(name="vchunk", bufs=2) as vchunk,
                tc.tile_pool(name="opool", bufs=2) as opool,
            ):
                # ---- K^T prefetch: one 8MB tile, 8 chunk-major 1MB DMAs
                # (the order S consumes it: S row tile 0 streams chunk by
                # chunk). The V bounce + gather are slotted in after chunk 2
                # so the V transfer overlaps the prefetch tail and the whole
                # S phase, finishing just before PV needs it — while the
                # prefetch head (which paces S row tile 0) stays
                # collective-free. ----
                kall = kallp.tile([P, n_dt, nq], f16, tag="kall", name="kall")
                half = nq // 2
                bv = bounce_v.rearrange("(jjt p) e -> p jjt e", p=P)
                for rr in range(cores):
                    rsl = slice(rr * sh, (rr + 1) * sh)
                    nc.sync.dma_start(kall[:, :, rsl], gk[rr])
                # V bounce + gather go out only after the whole prefetch has
                # been queued: an active collective starves shared-DRAM
                # reads, and the prefetch paces S row tile 0. The V transfer
                # then overlaps S row tiles 1-3 and finishes just before PV.
                nc.sync.dma_start(bv[:], v_loc[:])
                nc.gpsimd.collective_compute(
                    "AllGather", mybir.AluOpType.bypass,
                    replica_groups=rg,
                    ins=[bounce_v.ap().opt()],
                    outs=[gath_v.ap().opt()],
                )

                # s_sb holds RAW scores (std ~2.6e5 — far beyond fp16 max, so
                # f32). Only 2 bufs: s_sb[it] is dead once exp(it) has run.
                s_sb = [srow.tile([P, nq], f32, tag="s", name="s") for _ in range(n_it)]
                p_sb = [prow.tile([P, nq], f16, tag="p", name="p") for _ in range(n_it)]
                pt = [
                    ptp.tile([P, n_jt, P], f16, tag=f"pt{it}", name=f"pt{it}")
                    for it in range(n_it)
                ]

                # ---- S = Q K^T, it-outer so softmax + P^T of row tile it
                # overlap S of it+1. Row tile 0 is rr-middle/dtt-inner so it
                # consumes the gathered chunks at 1MB granularity as the
                # prefetch streams them in; row tiles 1-3 (chunks resident)
                # run dtt-middle over 4-chunk groups so one stationary Q^T
                # load feeds 4 matmuls (explicit LDWEIGHTS).
                def s_stats(it, rr, ps):
                    if rr == 0:
                        nc.vector.reduce_max(
                            m_t[it][:], ps[:], axis=mybir.AxisListType.X
                        )
                    else:
                        nc.vector.reduce_max(
                            tmpmax[:], ps[:], axis=mybir.AxisListType.X
                        )
                        nc.vector.tensor_max(m_t[it][:], m_t[it][:], tmpmax[:])
                    copy_eng(rr)(s_sb[it][:, rr * sh:(rr + 1) * sh], ps[:])

                _spsum_cm = tc.tile_pool(name="spsum", bufs=8, space="PSUM")
                spsum = _spsum_cm.__enter__()
                for it in range(n_it):
                    isl = slice(it * P, (it + 1) * P)
                    if it == 0:
                        for rr in range(cores):
                            rsl = slice(rr * sh, (rr + 1) * sh)
                            ps = spsum.tile([P, sh], f32, tag="sps", name="sps")
                            for dtt in range(n_dt):
                                nc.tensor.matmul(
                                    ps[:], qthi[:, dtt, isl], kall[:, dtt, rsl],
                                    start=(dtt == 0), stop=(dtt == n_dt - 1),
                                )
                            s_stats(it, rr, ps)
                    else:
                        for g in range(2):
                            chunks = range(4 * g, 4 * g + 4)
                            pss = {
                                c: spsum.tile([P, sh], f32, tag="sps", name="sps")
                                for c in chunks
                            }
                            for dtt in range(n_dt):
                                mm = group_ldw(qthi[:, dtt, isl])
                                for c in chunks:
                                    csl = slice(c * sh, (c + 1) * sh)
                                    mm(
                                        pss[c][:], qthi[:, dtt, isl],
                                        kall[:, dtt, csl],
                                        start=(dtt == 0), stop=(dtt == n_dt - 1),
                                    )
                            for c in chunks:
                                s_stats(it, c, pss[c])
                    # softmax for this row tile; P^T in two halves (on the
                    # sync queue, idle once the prefetch drains) so PV can
                    # start after the first halves land
                    nc.vector.tensor_scalar_mul(bias_t[it][:], m_t[it][:], -scale)
                    nc.scalar.activation(
                        p_sb[it][:], s_sb[it][:],
                        mybir.ActivationFunctionType.Exp,
                        bias=bias_t[it][:], scale=scale,
                        accum_out=ell_t[it][:],
                    )
                    nc.vector.reciprocal(rl_t[it][:], ell_t[it][:])
                    nc.sync.dma_start_transpose(
                        pt[it][:, : n_jt // 2], p_sb[it][:, :half]
                    )
                    nc.sync.dma_start_transpose(
                        pt[it][:, n_jt // 2:], p_sb[it][:, half:]
                    )
                _spsum_cm.__exit__(None, None, None)

                # ---- O = (P @ V) / ell, all 8 PSUM banks, single V pass.
                # eh pairs share the stationary P^T tile (LDWEIGHTS elided).
                _pvpsum_cm = tc.tile_pool(name="pvpsum", bufs=n_it * n_eh, space="PSUM")
                pvpsum = _pvpsum_cm.__enter__()
                pso = {
                    (it, eh): pvpsum.tile([P, EH], f32, tag="pvps", name="pvps")
                    for it in range(n_it) for eh in range(n_eh)
                }
                for jg in range(n_jg):
                    vc = vchunk.tile([P, JG, dim], f16, tag="vc", name="vc")
                    for jj in range(JG):
                        nc.scalar.dma_start(vc[:, jj], gv[jg][:, jj])
                    last = jg == n_jg - 1
                    for it in range(n_it):
                        for jj in range(JG):
                            mm = group_ldw(pt[it][:, jg * JG + jj])
                            for eh in range(n_eh):
                                esl = slice(eh * EH, (eh + 1) * EH)
                                mm(
                                    pso[(it, eh)][:],
                                    pt[it][:, jg * JG + jj],
                                    vc[:, jj, esl],
                                    start=(jg == 0 and jj == 0),
                                    stop=(last and jj == JG - 1),
                                )
                        if last:
                            # scale + store this row tile while the PE is
                            # still accumulating the remaining row tiles
                            o_sb = opool.tile([P, dim], f32, tag="o", name="o")
                            for eh in range(n_eh):
                                esl = slice(eh * EH, (eh + 1) * EH)
                                nc.vector.tensor_scalar_mul(
                                    o_sb[:, esl], pso[(it, eh)][:], rl_t[it][:]
                                )
                            nc.scalar.dma_start(
                                out_ext[it * P:(it + 1) * P, :], o_sb[:]
                            )
                _pvpsum_cm.__exit__(None, None, None)

    return nc


_CACHE = {}
RUN_KW = {}


def _get_nc():
    if "nc" not in _CACHE:
        _CACHE["nc"] = build_attention()
    return _CACHE["nc"]


def kernel(**inputs):
    from concourse.bass_utils import run_bass_kernel_spmd

    q = np.ascontiguousarray(np.asarray(inputs["q"], dtype=np.float32))
    k = np.ascontiguousarray(np.asarray(inputs["k"], dtype=np.float32))
    v = np.ascontiguousarray(np.asarray(inputs["v"], dtype=np.float32))
    W_q = np.ascontiguousarray(np.asarray(inputs["W_q"], dtype=np.float32))
    W_k = np.ascontiguousarray(np.asarray(inputs["W_k"], dtype=np.float32))
    W_v = np.ascontiguousarray(np.asarray(inputs["W_v"], dtype=np.float32))

    sh = N_Q // CORES
    in_maps = []
    for r in range(CORES):
        sl = slice(r * sh, (r + 1) * sh)
        in_maps.append({
            "q": q[sl], "k": k[sl], "v": v[sl],
            "W_q": W_q, "W_k": W_k, "W_v": W_v,
        })

    nc = _get_nc()
    if not nc.is_finalized():
        nc.finalize()
    res = run_bass_kernel_spmd(nc, in_maps, core_ids=list(range(CORES)), **RUN_KW)
    _CACHE["last_result"] = res
    out = np.concatenate([res.results[r]["out"] for r in range(CORES)], axis=0)
    return out


if __name__ == "__main__":
    import reference

    inputs = {kk: np.asarray(vv) for kk, vv in reference.setup_inputs().items()}
    out = kernel(**inputs)
    print("out shape:", out.shape, out.dtype)


# revision 33
# speedup vs baseline: 1.0281x; 1.0281x over previous
"""Distributed attention layer kernel for 8 TRN2 NeuronCores.

Reference computation (f32):
    Q = q @ W_q; K = k @ W_k; V = v @ W_v
    out = softmax((Q @ K^T)/sqrt(d_k)) @ V

Sharding: rows of q/k/v are split 8 ways (sequence parallel). Each core
projects its own shards, the K^T/V projections are all-gathered (fp16),
and each core computes its 512-row slice of the attention output.

v5 restructure (from the traces of the v1-v4 kernels):
 - K path is minimal-latency: per-ct weight loads, it-major transposes
   (no mid-burst stalls), ct-outer two-pass projection (4 PSUM
   banks/pass), one half-tensor bounce per pass (DMA trigger cost on
   the Activation sequencer is ~0.7us each — fewer, bigger bounces),
   for the earliest possible K all-gather trigger. The gather end is
   gated by (launch skew + last core's K path).
 - V path runs before Q path; its bounce + gather are issued on the
   sync queue after the whole K^T prefetch (an active collective
   starves shared-DRAM reads), so the V transfer overlaps S row tiles
   1-3 and completes just before PV needs it.
 - S phase is it-outer (softmax + P^T of row tile it overlap S of
   it+1, removing the ~22us S->PV bubble). Row tile 0 is
   rr-middle/dtt-inner, streaming the gathered chunks at 1MB
   granularity; row tiles 1-3 run 4-chunk groups with one explicit
   LDWEIGHTS per stationary Q^T tile.
 - PV pairs each stationary P^T tile's two eh matmuls behind one
   explicit LDWEIGHTS.
 - Engine/queue split in S+PV: PSUM copies alternate Activation/DVE,
   reductions on DVE, exp on Activation, P^T half-transposes on the
   sync queue, V-chunk loads + output stores on the Activation queue.
 - Scores staging stays f32 (raw scores reach ~2.6e5, beyond fp16 max)
   but in a 2-buffer rotation: s_sb[it] is dead once exp(it) has run.
 - NOTE: bare ldweights=False WITHOUT a standalone InstLdweights
   partner is broken on HW (walrus ignores it for fp16 and emits
   garbage for f32r); only the explicit pairing below is safe.

Precision: projections run in f32r (full rate for free-dim >= 256) with
f32 PSUM accumulation; attention matmuls are fp16 with f32 accumulation;
softmax is f32 ACT exp with per-row max bias. Measured end-to-end error
vs the f32 reference: ~8e-3 (gate 2e-2).
"""

import os
import sys

for _p in ("/opt/pypackages", "/opt/trn_rl_repo"):
    if _p not in sys.path:
        sys.path.insert(0, _p)

import numpy as np

N_Q, N_KV, DIM = 4096, 4096, 1024  # D_K = D_V = DIM (square weights)
CORES = 8

P = 128

# Emit explicit InstLdweights + non-self-loading matmuls (fp16 only)
# where one stationary tile feeds several consecutive matmuls. The
# fused LDWEIGHTS+MATMUL pair measures 263ns for a 512-row fp16 matmul
# vs the 213ns matmul floor; explicit pairing recovers most of that 19%
# PE tax in S (4x reuse) and PV (2x reuse). (Setting ldweights=False
# WITHOUT a standalone InstLdweights partner is broken: walrus ignores
# it for fp16 and emits garbage for f32r.)
EXPLICIT_LDW = False


def build_attention(nq=N_Q, dim=DIM, cores=CORES):
    """Build the per-core Bass graph (SPMD; identical on all cores)."""
    import concourse.bass as bass
    import concourse.mybir as mybir
    from concourse import bacc
    from concourse.masks import make_identity
    from concourse.tile import TileContext

    dt = mybir.dt
    f32, f32r, f16 = dt.float32, dt.float32r, dt.float16

    sh = nq // cores          # rows per core (512)
    n_ct = dim // P           # contraction tiles for projections (8)
    n_dt = dim // P           # d tiles (8)
    n_it = sh // P            # query-row tiles per core (4)
    n_jt = nq // P            # total kv j tiles (32)
    JG = 4                    # j-tiles per PV V-chunk
    n_jg = n_jt // JG         # V chunk count (8)
    EH = 512
    n_eh = dim // EH          # 512-wide output column halves (2)
    scale = 1.0 / float(np.sqrt(dim))

    nc = bacc.Bacc(num_devices=cores)

    # --- external I/O (per core: row shards of q/k/v, full weights) ---
    q_ext = nc.declare_dram_parameter("q", [sh, dim], f32, isOutput=False)
    k_ext = nc.declare_dram_parameter("k", [sh, dim], f32, isOutput=False)
    v_ext = nc.declare_dram_parameter("v", [sh, dim], f32, isOutput=False)
    wq_ext = nc.declare_dram_parameter("W_q", [dim, dim], f32r, isOutput=False)
    wk_ext = nc.declare_dram_parameter("W_k", [dim, dim], f32r, isOutput=False)
    wv_ext = nc.declare_dram_parameter("W_v", [dim, dim], f32r, isOutput=False)
    out_ext = nc.declare_dram_parameter("out", [sh, dim], f32, isOutput=True)

    # --- internal DRAM for collectives ---
    bounce_k = nc.dram_tensor("bounce_k", [dim, sh], f16)
    bounce_v = nc.dram_tensor("bounce_v", [sh, dim], f16)
    gath_k = nc.dram_tensor("gath_k", [cores * dim, sh], f16, addr_space="Shared")
    gath_v = nc.dram_tensor("gath_v", [cores * sh, dim], f16, addr_space="Shared")

    rg = [list(range(cores))]

    def group_ldw(stationary):
        """Load a stationary fp16 tile once for the following group of
        matmuls. Returns a function wrapping nc.tensor.matmul that marks
        the matmul non-self-loading (pairs with the explicit load)."""
        if EXPLICIT_LDW:
            nc.tensor.ldweights(stationary)

            def mm(*a, **kw):
                bi = nc.tensor.matmul(*a, **kw)
                bi.ins.ldweights = False
                return bi
            return mm
        return nc.tensor.matmul

    with TileContext(nc) as tc:
        with (
            tc.tile_pool(name="const", bufs=1) as constp,
            tc.tile_pool(name="qt", bufs=1) as qtp,
            tc.tile_pool(name="stats", bufs=1) as statp,
        ):
            # NOTE: make_identity/PE-transpose on float32r crashes walrus
            # codegen; transposes run in plain f32 and the psum result is
            # copy-cast (bit-identical) into float32r SBUF tiles.
            ident_f = constp.tile([P, P], f32, tag="idf", name="idf")
            make_identity(nc, ident_f)

            qthi = qtp.tile([P, n_dt, sh], f16, tag="qthi", name="qthi")
            v_loc = qtp.tile([P, sh // P, dim], f16, tag="v_loc", name="v_loc")

            with (
                tc.tile_pool(name="w", bufs=1) as wpool,
                tc.tile_pool(name="iost", bufs=6) as iost,
                tc.tile_pool(name="tin", bufs=2) as tpool,
                tc.tile_pool(name="kvout", bufs=1) as kvout,
                tc.tile_pool(name="tpsum", bufs=4, space="PSUM") as tpsum,
                tc.tile_pool(name="ppsum", bufs=4, space="PSUM") as ppsum,
            ):
                # Bulk loads stream in K-path-first order on the sync (SP)
                # HWDGE queue; weights load per-ct so the ct-outer
                # projections can start before the full 4MB arrives. The
                # Activation HWDGE queue is reserved for latency-critical
                # transfers (bounce tiles, P^T XBAR transposes, outputs).
                def load_input(x_ext):
                    stgs = []
                    xsrc = x_ext.rearrange("(it p) c -> p it c", p=P)
                    for it in range(sh // P):
                        stg = iost.tile([P, dim], f32, tag="iostg", name="iostg")
                        nc.sync.dma_start(stg[:], xsrc[:, it])
                        stgs.append(stg)
                    return stgs

                wk = wpool.tile([P, n_ct, dim], f32r, tag="wk", name="wk")
                wv = wpool.tile([P, n_ct, dim], f32r, tag="wv", name="wv")
                wq = wpool.tile([P, n_ct, dim], f32r, tag="wq", name="wq")
                wk_src = wk_ext.rearrange("(ct p) d -> p ct d", p=P)
                wq_src = wq_ext.rearrange("(ct p) d -> p ct d", p=P)
                wv_src = wv_ext.rearrange("(ct p) d -> p ct d", p=P)

                # interleave k row tiles with the first wk column tiles so
                # the ct-outer K projection's weights arrive right behind
                # the transposes instead of after the whole k shard
                k_stg = []
                ksrc = k_ext.rearrange("(it p) c -> p it c", p=P)
                for it in range(sh // P):
                    stg = iost.tile([P, dim], f32, tag="iostg", name="iostg")
                    nc.sync.dma_start(stg[:], ksrc[:, it])
                    k_stg.append(stg)
                    nc.sync.dma_start(wk[:, it], wk_src[:, it])
                for ct in range(sh // P, n_ct):
                    nc.sync.dma_start(wk[:, ct], wk_src[:, ct])
                v_stg = load_input(v_ext)
                for ct in range(n_ct):
                    nc.sync.dma_start(wv[:, ct], wv_src[:, ct])
                q_stg = load_input(q_ext)
                for ct in range(n_ct):
                    nc.sync.dma_start(wq[:, ct], wq_src[:, ct])

                def transpose_input(stgs, tag):
                    """Transpose a staged [sh, dim] f32 input on the PE into a
                    [c_in=128, ct, row] f32r SBUF tile (copy-cast from psum).
                    it-major: each staged row tile is consumed in one 8-long
                    back-to-back burst as it lands, so the PE never stalls
                    (stalls reset the pstate ramp)."""
                    xt = tpool.tile([P, n_ct, sh], f32r, tag=tag, name=tag)
                    for it, stg in enumerate(stgs):
                        dst = slice(it * P, (it + 1) * P)
                        for ct in range(n_ct):
                            ps = tpsum.tile([P, P], f32, tag="tps", name="tps")
                            nc.tensor.transpose(
                                ps[:], stg[:, ct * P:(ct + 1) * P], ident_f
                            )
                            nc.vector.tensor_copy(xt[:, ct, dst], ps[:])
                    return xt

                def copy_eng(i):
                    return nc.scalar.copy if i % 2 == 0 else nc.vector.tensor_copy

                # ---- K path first: project K^T ct-outer in two 4-bank
                # passes, bounce each dtt tile as its copy lands, then
                # all-gather. The gather end is gated by the LAST core's
                # trigger (launch skew), so every us saved here moves the
                # whole S phase earlier. ----
                kt = transpose_input(k_stg, "xt")
                kt_loc = kvout.tile([P, n_dt, sh], f16, tag="kt_loc", name="kt_loc")
                bk = bounce_k.rearrange("(dtt p) jj -> p dtt jj", p=P)

                def project_dt(w_t, x_t, out_cb):
                    """out[dtt] = (W^T X^T)[dtt] for all 8 dtt column tiles,
                    two ct-outer passes of 4 PSUM banks each. out_cb(dtt, ps)
                    consumes the finished [P, sh] psum tile."""
                    for g in range(2):
                        dts = range(4 * g, 4 * g + 4)
                        pss = {
                            dtt: ppsum.tile([P, sh], f32, tag="pps", name="pps")
                            for dtt in dts
                        }
                        for ct in range(n_ct):
                            for dtt in dts:
                                dsl = slice(dtt * P, (dtt + 1) * P)
                                nc.tensor.matmul(
                                    pss[dtt][:], w_t[:, ct, dsl], x_t[:, ct],
                                    start=(ct == 0), stop=(ct == n_ct - 1),
                                )
                        for i, dtt in enumerate(dts):
                            out_cb(i, dtt, pss[dtt])

                def k_out(i, dtt, ps):
                    copy_eng(i)(kt_loc[:, dtt], ps[:])
                    if dtt in (3, 7):
                        # one half-tensor bounce per 4-bank pass: a single
                        # ~0.7us DMA trigger instead of four (the Activation
                        # sequencer cost per trigger, not the bytes, is what
                        # delays the gather trigger)
                        hs = slice(dtt - 3, dtt + 1)
                        nc.scalar.dma_start(bk[:, hs], kt_loc[:, hs])

                project_dt(wk, kt, k_out)
                nc.gpsimd.collective_compute(
                    "AllGather", mybir.AluOpType.bypass, replica_groups=rg,
                    ins=[bounce_k.ap().opt()], outs=[gath_k.ap().opt()],
                )

                # ---- V path second: project the V shard (jjt-pair passes).
                # No bounce here: the V all-gather's DRAM traffic would
                # starve the K^T chunk prefetch right after the K gather, so
                # the bounce DMA is issued on the sync queue mid-prefetch
                # (see the attention phase below). ----
                vt = transpose_input(v_stg, "xt")
                for g in range(2):
                    jjts = range(2 * g, 2 * g + 2)
                    pss = {
                        (jjt, eh): ppsum.tile([P, EH], f32, tag="pps", name="pps")
                        for jjt in jjts for eh in range(n_eh)
                    }
                    for ct in range(n_ct):
                        for jjt in jjts:
                            jsl = slice(jjt * P, (jjt + 1) * P)
                            for eh in range(n_eh):
                                esl = slice(eh * EH, (eh + 1) * EH)
                                nc.tensor.matmul(
                                    pss[(jjt, eh)][:], vt[:, ct, jsl],
                                    wv[:, ct, esl],
                                    start=(ct == 0), stop=(ct == n_ct - 1),
                                )
                    for i, (jjt, eh) in enumerate(pss):
                        esl = slice(eh * EH, (eh + 1) * EH)
                        copy_eng(i)(v_loc[:, jjt, esl], pss[(jjt, eh)][:])

                # ---- Q path last (local only; needed first at S start) ----
                qt = transpose_input(q_stg, "xt")

                def q_out(i, dtt, ps):
                    copy_eng(i)(qthi[:, dtt], ps[:])

                project_dt(wq, qt, q_out)

            # ================= attention phase =================
            m_t = [statp.tile([P, 1], f32, tag=f"m{it}", name=f"m{it}") for it in range(n_it)]
            tmpmax = statp.tile([P, 1], f32, tag="tmpmax", name="tmpmax")
            bias_t = [statp.tile([P, 1], f32, tag=f"b{it}", name=f"b{it}") for it in range(n_it)]
            ell_t = [statp.tile([P, 1], f32, tag=f"l{it}", name=f"l{it}") for it in range(n_it)]
            rl_t = [statp.tile([P, 1], f32, tag=f"r{it}", name=f"r{it}") for it in range(n_it)]

            gk = gath_k.rearrange("(r dtt p) jj -> r p dtt jj", r=cores, p=P)
            gv = gath_v.rearrange("(jg jj p) e -> jg p jj e", jj=JG, p=P)

            with (
                tc.tile_pool(name="kall", bufs=1) as kallp,
                tc.tile_pool(name="srow", bufs=2) as srow,
                tc.tile_pool(name="prow", bufs=2) as prow,
                tc.tile_pool(name="ptp", bufs=1) as ptp,
                tc.tile_pool(name="vchunk", bufs=2) as vchunk,
                tc.tile_pool(name="opool", bufs=2) as opool,
            ):
                # ---- K^T prefetch: one 8MB tile, 8 chunk-major 1MB DMAs
                # (the order S consumes it: S row tile 0 streams chunk by
                # chunk). The V bounce + gather are slotted in after chunk 2
                # so the V transfer overlaps the prefetch tail and the whole
                # S phase, finishing just before PV needs it — while the
                # prefetch head (which paces S row tile 0) stays
                # collective-free. ----
                kall = kallp.tile([P, n_dt, nq], f16, tag="kall", name="kall")
                half = nq // 2
                bv = bounce_v.rearrange("(jjt p) e -> p jjt e", p=P)
                for rr in range(cores):
                    rsl = slice(rr * sh, (rr + 1) * sh)
                    nc.sync.dma_start(kall[:, :, rsl], gk[rr])


                # s_sb holds RAW scores (std ~2.6e5 — far beyond fp16 max, so
                # f32). Only 2 bufs: s_sb[it] is dead once exp(it) has run.
                s_sb = [srow.tile([P, nq], f32, tag="s", name="s") for _ in range(n_it)]
                p_sb = [prow.tile([P, nq], f16, tag="p", name="p") for _ in range(n_it)]
                pt = [
                    ptp.tile([P, n_jt, P], f16, tag=f"pt{it}", name=f"pt{it}")
                    for it in range(n_it)
                ]

                # ---- S = Q K^T, it-outer so softmax + P^T of row tile it
                # overlap S of it+1. Row tile 0 is rr-middle/dtt-inner so it
                # consumes the gathered chunks at 1MB granularity as the
                # prefetch streams them in; row tiles 1-3 (chunks resident)
                # run dtt-middle over 4-chunk groups so one stationary Q^T
                # load feeds 4 matmuls (explicit LDWEIGHTS).
                def s_stats(it, rr, ps):
                    if rr == 0:
                        nc.vector.reduce_max(
                            m_t[it][:], ps[:], axis=mybir.AxisListType.X
                        )
                    else:
                        nc.vector.reduce_max(
                            tmpmax[:], ps[:], axis=mybir.AxisListType.X
                        )
                        nc.vector.tensor_max(m_t[it][:], m_t[it][:], tmpmax[:])
                    copy_eng(rr)(s_sb[it][:, rr * sh:(rr + 1) * sh], ps[:])

                _spsum_cm = tc.tile_pool(name="spsum", bufs=8, space="PSUM")
                spsum = _spsum_cm.__enter__()
                for it in range(n_it):
                    isl = slice(it * P, (it + 1) * P)
                    if it == 0:
                        for rr in range(cores):
                            rsl = slice(rr * sh, (rr + 1) * sh)
                            ps = spsum.tile([P, sh], f32, tag="sps", name="sps")
                            for dtt in range(n_dt):
                                nc.tensor.matmul(
                                    ps[:], qthi[:, dtt, isl], kall[:, dtt, rsl],
                                    start=(dtt == 0), stop=(dtt == n_dt - 1),
                                )
                            s_stats(it, rr, ps)
                            if rr == 5:
                                # V bounce rides the scalar queue behind the
                                # rr==4 copy: it executes only once row tile
                                # 0 has consumed most of the K^T prefetch,
                                # so the V collective's DRAM traffic never
                                # starves the prefetch head — yet V still
                                # completes well before PV needs it.
                                nc.scalar.dma_start(bv[:], v_loc[:])
                                nc.gpsimd.collective_compute(
                                    "AllGather", mybir.AluOpType.bypass,
                                    replica_groups=rg,
                                    ins=[bounce_v.ap().opt()],
                                    outs=[gath_v.ap().opt()],
                                )
                    else:
                        for g in range(2):
                            chunks = range(4 * g, 4 * g + 4)
                            pss = {
                                c: spsum.tile([P, sh], f32, tag="sps", name="sps")
                                for c in chunks
                            }
                            for dtt in range(n_dt):
                                mm = group_ldw(qthi[:, dtt, isl])
                                for c in chunks:
                                    csl = slice(c * sh, (c + 1) * sh)
                                    mm(
                                        pss[c][:], qthi[:, dtt, isl],
                                        kall[:, dtt, csl],
                                        start=(dtt == 0), stop=(dtt == n_dt - 1),
                                    )
                            for c in chunks:
                                s_stats(it, c, pss[c])
                    # softmax for this row tile; P^T in two halves (on the
                    # sync queue, idle once the prefetch drains) so PV can
                    # start after the first halves land
                    nc.vector.tensor_scalar_mul(bias_t[it][:], m_t[it][:], -scale)
                    nc.scalar.activation(
                        p_sb[it][:], s_sb[it][:],
                        mybir.ActivationFunctionType.Exp,
                        bias=bias_t[it][:], scale=scale,
                        accum_out=ell_t[it][:],
                    )
                    nc.vector.reciprocal(rl_t[it][:], ell_t[it][:])
                    nc.sync.dma_start_transpose(
                        pt[it][:, : n_jt // 2], p_sb[it][:, :half]
                    )
                    nc.sync.dma_start_transpose(
                        pt[it][:, n_jt // 2:], p_sb[it][:, half:]
                    )
                _spsum_cm.__exit__(None, None, None)

                # ---- O = (P @ V) / ell, all 8 PSUM banks, single V pass.
                # eh pairs share the stationary P^T tile (LDWEIGHTS elided).
                _pvpsum_cm = tc.tile_pool(name="pvpsum", bufs=n_it * n_eh, space="PSUM")
                pvpsum = _pvpsum_cm.__enter__()
                pso = {
                    (it, eh): pvpsum.tile([P, EH], f32, tag="pvps", name="pvps")
                    for it in range(n_it) for eh in range(n_eh)
                }
                for jg in range(n_jg):
                    vc = vchunk.tile([P, JG, dim], f16, tag="vc", name="vc")
                    for jj in range(JG):
                        nc.scalar.dma_start(vc[:, jj], gv[jg][:, jj])
                    last = jg == n_jg - 1
                    for it in range(n_it):
                        for jj in range(JG):
                            mm = group_ldw(pt[it][:, jg * JG + jj])
                            for eh in range(n_eh):
                                esl = slice(eh * EH, (eh + 1) * EH)
                                mm(
                                    pso[(it, eh)][:],
                                    pt[it][:, jg * JG + jj],
                                    vc[:, jj, esl],
                                    start=(jg == 0 and jj == 0),
                                    stop=(last and jj == JG - 1),
                                )
                        if last:
                            # scale + store this row tile while the PE is
                            # still accumulating the remaining row tiles
                            o_sb = opool.tile([P, dim], f32, tag="o", name="o")
                            for eh in range(n_eh):
                                esl = slice(eh * EH, (eh + 1) * EH)
                                nc.vector.tensor_scalar_mul(
                                    o_sb[:, esl], pso[(it, eh)][:], rl_t[it][:]
                                )
                            nc.scalar.dma_start(
                                out_ext[it * P:(it + 1) * P, :], o_sb[:]
                            )
                _pvpsum_cm.__exit__(None, None, None)

    return nc


_CACHE = {}
RUN_KW = {}


def _get_nc():
    if "nc" not in _CACHE:
        _CACHE["nc"] = build_attention()
    return _CACHE["nc"]


def kernel(**inputs):
    from concourse.bass_utils import run_bass_kernel_spmd

    q = np.ascontiguousarray(np.asarray(inputs["q"], dtype=np.float32))
    k = np.ascontiguousarray(np.asarray(inputs["k"], dtype=np.float32))
    v = np.ascontiguousarray(np.asarray(inputs["v"], dtype=np.float32))
    W_q = np.ascontiguousarray(np.asarray(inputs["W_q"], dtype=np.float32))
    W_k = np.ascontiguousarray(np.asarray(inputs["W_k"], dtype=np.float32))
    W_v = np.ascontiguousarray(np.asarray(inputs["W_v"], dtype=np.float32))

    sh = N_Q // CORES
    in_maps = []
    for r in range(CORES):
        sl = slice(r * sh, (r + 1) * sh)
        in_maps.append({
            "q": q[sl], "k": k[sl], "v": v[sl],
            "W_q": W_q, "W_k": W_k, "W_v": W_v,
        })

    nc = _get_nc()
    if not nc.is_finalized():
        nc.finalize()
    res = run_bass_kernel_spmd(nc, in_maps, core_ids=list(range(CORES)), **RUN_KW)
    _CACHE["last_result"] = res
    out = np.concatenate([res.results[r]["out"] for r in range(CORES)], axis=0)
    return out


if __name__ == "__main__":
    import reference

    inputs = {kk: np.asarray(vv) for kk, vv in reference.setup_inputs().items()}
    out = kernel(**inputs)
    print("out shape:", out.shape, out.dtype)


# revision 37
# speedup vs baseline: 1.1386x; 1.1075x over previous
"""Distributed attention layer kernel for 8 TRN2 NeuronCores.

Reference computation (f32):
    Q = q @ W_q; K = k @ W_k; V = v @ W_v
    out = softmax((Q @ K^T)/sqrt(d_k)) @ V

Sharding: rows of q/k/v are split 8 ways (sequence parallel). Each core
projects its own shards, the K^T/V projections are all-gathered (fp16),
and each core computes its 512-row slice of the attention output.

v5 restructure (from the traces of the v1-v4 kernels):
 - K path is minimal-latency: per-ct weight loads, it-major transposes
   (no mid-burst stalls), ct-outer two-pass projection (4 PSUM
   banks/pass), one half-tensor bounce per pass (DMA trigger cost on
   the Activation sequencer is ~0.7us each — fewer, bigger bounces),
   for the earliest possible K all-gather trigger. The gather end is
   gated by (launch skew + last core's K path).
 - V path runs before Q path; its bounce + gather are issued on the
   sync queue after the whole K^T prefetch (an active collective
   starves shared-DRAM reads), so the V transfer overlaps S row tiles
   1-3 and completes just before PV needs it.
 - S phase is it-outer (softmax + P^T of row tile it overlap S of
   it+1, removing the ~22us S->PV bubble). Row tile 0 is
   rr-middle/dtt-inner, streaming the gathered chunks at 1MB
   granularity; row tiles 1-3 run 4-chunk groups with one explicit
   LDWEIGHTS per stationary Q^T tile.
 - PV pairs each stationary P^T tile's two eh matmuls behind one
   explicit LDWEIGHTS.
 - Engine/queue split in S+PV: PSUM copies alternate Activation/DVE,
   reductions on DVE, exp on Activation, P^T half-transposes on the
   sync queue, V-chunk loads + output stores on the Activation queue.
 - Scores staging stays f32 (raw scores reach ~2.6e5, beyond fp16 max)
   but in a 2-buffer rotation: s_sb[it] is dead once exp(it) has run.
 - NOTE: bare ldweights=False WITHOUT a standalone InstLdweights
   partner is broken on HW (walrus ignores it for fp16 and emits
   garbage for f32r); only the explicit pairing below is safe.

Precision: projections run in f32r (full rate for free-dim >= 256) with
f32 PSUM accumulation; attention matmuls are fp16 with f32 accumulation;
softmax is f32 ACT exp with per-row max bias. Measured end-to-end error
vs the f32 reference: ~8e-3 (gate 2e-2).
"""

import os
import sys

for _p in ("/opt/pypackages", "/opt/trn_rl_repo"):
    if _p not in sys.path:
        sys.path.insert(0, _p)

import numpy as np

N_Q, N_KV, DIM = 4096, 4096, 1024  # D_K = D_V = DIM (square weights)
CORES = 8

P = 128

# Emit explicit InstLdweights + non-self-loading matmuls (fp16 only)
# where one stationary tile feeds several consecutive matmuls. The
# fused LDWEIGHTS+MATMUL pair measures 263ns for a 512-row fp16 matmul
# vs the 213ns matmul floor; explicit pairing recovers most of that 19%
# PE tax in S (4x reuse) and PV (2x reuse). (Setting ldweights=False
# WITHOUT a standalone InstLdweights partner is broken: walrus ignores
# it for fp16 and emits garbage for f32r.)
EXPLICIT_LDW = False


def build_attention(nq=N_Q, dim=DIM, cores=CORES):
    """Build the per-core Bass graph (SPMD; identical on all cores)."""
    import concourse.bass as bass
    import concourse.mybir as mybir
    from concourse import bacc
    from concourse.masks import make_identity
    from concourse.tile import TileContext

    dt = mybir.dt
    f32, f32r, f16 = dt.float32, dt.float32r, dt.float16

    sh = nq // cores          # rows per core (512)
    n_ct = dim // P           # contraction tiles for projections (8)
    n_dt = dim // P           # d tiles (8)
    n_it = sh // P            # query-row tiles per core (4)
    n_jt = nq // P            # total kv j tiles (32)
    JG = 4                    # j-tiles per PV V-chunk
    n_jg = n_jt // JG         # V chunk count (8)
    EH = 512
    n_eh = dim // EH          # 512-wide output column halves (2)
    scale = 1.0 / float(np.sqrt(dim))

    nc = bacc.Bacc(num_devices=cores)

    # --- external I/O (per core: row shards of q/k/v, full weights) ---
    q_ext = nc.declare_dram_parameter("q", [sh, dim], f32, isOutput=False)
    k_ext = nc.declare_dram_parameter("k", [sh, dim], f32, isOutput=False)
    v_ext = nc.declare_dram_parameter("v", [sh, dim], f32, isOutput=False)
    wq_ext = nc.declare_dram_parameter("W_q", [dim, dim], f32r, isOutput=False)
    wk_ext = nc.declare_dram_parameter("W_k", [dim, dim], f32r, isOutput=False)
    wv_ext = nc.declare_dram_parameter("W_v", [dim, dim], f32r, isOutput=False)
    out_ext = nc.declare_dram_parameter("out", [sh, dim], f32, isOutput=True)

    # --- internal DRAM for collectives ---
    bounce_k = nc.dram_tensor("bounce_k", [dim, sh], f16)
    bounce_v = nc.dram_tensor("bounce_v", [sh, dim], f16)
    gath_k = nc.dram_tensor("gath_k", [cores * dim, sh], f16, addr_space="Shared")
    gath_v = nc.dram_tensor("gath_v", [cores * sh, dim], f16, addr_space="Shared")

    rg = [list(range(cores))]

    def group_ldw(stationary):
        """Load a stationary fp16 tile once for the following group of
        matmuls. Returns a function wrapping nc.tensor.matmul that marks
        the matmul non-self-loading (pairs with the explicit load)."""
        if EXPLICIT_LDW:
            nc.tensor.ldweights(stationary)

            def mm(*a, **kw):
                bi = nc.tensor.matmul(*a, **kw)
                bi.ins.ldweights = False
                return bi
            return mm
        return nc.tensor.matmul

    with TileContext(nc) as tc:
        with (
            tc.tile_pool(name="const", bufs=1) as constp,
            tc.tile_pool(name="qt", bufs=1) as qtp,
            tc.tile_pool(name="stats", bufs=1) as statp,
        ):
            # NOTE: make_identity/PE-transpose on float32r crashes walrus
            # codegen; transposes run in plain f32 and the psum result is
            # copy-cast (bit-identical) into float32r SBUF tiles.
            ident_f = constp.tile([P, P], f32, tag="idf", name="idf")
            make_identity(nc, ident_f)

            qthi = qtp.tile([P, n_dt, sh], f16, tag="qthi", name="qthi")
            v_loc = qtp.tile([P, sh // P, dim], f16, tag="v_loc", name="v_loc")

            with (
                tc.tile_pool(name="w", bufs=1) as wpool,
                tc.tile_pool(name="iost", bufs=6) as iost,
                tc.tile_pool(name="tin", bufs=2) as tpool,
                tc.tile_pool(name="kvout", bufs=1) as kvout,
                tc.tile_pool(name="tpsum", bufs=4, space="PSUM") as tpsum,
                tc.tile_pool(name="ppsum", bufs=4, space="PSUM") as ppsum,
            ):
                # Bulk loads stream in K-path-first order on the sync (SP)
                # HWDGE queue; weights load per-ct so the ct-outer
                # projections can start before the full 4MB arrives. The
                # Activation HWDGE queue is reserved for latency-critical
                # transfers (bounce tiles, P^T XBAR transposes, outputs).
                def load_input(x_ext):
                    stgs = []
                    xsrc = x_ext.rearrange("(it p) c -> p it c", p=P)
                    for it in range(sh // P):
                        stg = iost.tile([P, dim], f32, tag="iostg", name="iostg")
                        nc.sync.dma_start(stg[:], xsrc[:, it])
                        stgs.append(stg)
                    return stgs

                wk = wpool.tile([P, n_ct, dim], f32r, tag="wk", name="wk")
                wv = wpool.tile([P, n_ct, dim], f32r, tag="wv", name="wv")
                wq = wpool.tile([P, n_ct, dim], f32r, tag="wq", name="wq")
                wk_src = wk_ext.rearrange("(ct p) d -> p ct d", p=P)
                wq_src = wq_ext.rearrange("(ct p) d -> p ct d", p=P)
                wv_src = wv_ext.rearrange("(ct p) d -> p ct d", p=P)

                # interleave k row tiles with the first wk column tiles so
                # the ct-outer K projection's weights arrive right behind
                # the transposes instead of after the whole k shard
                k_stg = []
                ksrc = k_ext.rearrange("(it p) c -> p it c", p=P)
                for it in range(sh // P):
                    stg = iost.tile([P, dim], f32, tag="iostg", name="iostg")
                    nc.sync.dma_start(stg[:], ksrc[:, it])
                    k_stg.append(stg)
                    nc.sync.dma_start(wk[:, it], wk_src[:, it])
                for ct in range(sh // P, n_ct):
                    nc.sync.dma_start(wk[:, ct], wk_src[:, ct])
                v_stg = load_input(v_ext)
                for ct in range(n_ct):
                    nc.sync.dma_start(wv[:, ct], wv_src[:, ct])
                q_stg = load_input(q_ext)
                for ct in range(n_ct):
                    nc.sync.dma_start(wq[:, ct], wq_src[:, ct])

                def transpose_input(stgs, tag):
                    """Transpose a staged [sh, dim] f32 input on the PE into a
                    [c_in=128, ct, row] f32r SBUF tile (copy-cast from psum).
                    it-major: each staged row tile is consumed in one 8-long
                    back-to-back burst as it lands, so the PE never stalls
                    (stalls reset the pstate ramp)."""
                    xt = tpool.tile([P, n_ct, sh], f32r, tag=tag, name=tag)
                    for it, stg in enumerate(stgs):
                        dst = slice(it * P, (it + 1) * P)
                        for ct in range(n_ct):
                            ps = tpsum.tile([P, P], f32, tag="tps", name="tps")
                            nc.tensor.transpose(
                                ps[:], stg[:, ct * P:(ct + 1) * P], ident_f
                            )
                            nc.vector.tensor_copy(xt[:, ct, dst], ps[:])
                    return xt

                def copy_eng(i):
                    return nc.scalar.copy if i % 2 == 0 else nc.vector.tensor_copy

                # ---- K path first: project K^T ct-outer in two 4-bank
                # passes, bounce each dtt tile as its copy lands, then
                # all-gather. The gather end is gated by the LAST core's
                # trigger (launch skew), so every us saved here moves the
                # whole S phase earlier. ----
                kt = transpose_input(k_stg, "xt")
                kt_loc = kvout.tile([P, n_dt, sh], f16, tag="kt_loc", name="kt_loc")
                bk = bounce_k.rearrange("(dtt p) jj -> p dtt jj", p=P)

                def project_dt(w_t, x_t, out_cb):
                    """out[dtt] = (W^T X^T)[dtt] for all 8 dtt column tiles,
                    two ct-outer passes of 4 PSUM banks each. out_cb(dtt, ps)
                    consumes the finished [P, sh] psum tile."""
                    for g in range(2):
                        dts = range(4 * g, 4 * g + 4)
                        pss = {
                            dtt: ppsum.tile([P, sh], f32, tag="pps", name="pps")
                            for dtt in dts
                        }
                        for ct in range(n_ct):
                            for dtt in dts:
                                dsl = slice(dtt * P, (dtt + 1) * P)
                                nc.tensor.matmul(
                                    pss[dtt][:], w_t[:, ct, dsl], x_t[:, ct],
                                    start=(ct == 0), stop=(ct == n_ct - 1),
                                )
                        for i, dtt in enumerate(dts):
                            out_cb(i, dtt, pss[dtt])

                def k_out(i, dtt, ps):
                    copy_eng(i)(kt_loc[:, dtt], ps[:])
                    if dtt in (3, 7):
                        # one half-tensor bounce per 4-bank pass: a single
                        # ~0.7us DMA trigger instead of four (the Activation
                        # sequencer cost per trigger, not the bytes, is what
                        # delays the gather trigger)
                        hs = slice(dtt - 3, dtt + 1)
                        nc.scalar.dma_start(bk[:, hs], kt_loc[:, hs])

                project_dt(wk, kt, k_out)
                nc.gpsimd.collective_compute(
                    "AllGather", mybir.AluOpType.bypass, replica_groups=rg,
                    ins=[bounce_k.ap().opt()], outs=[gath_k.ap().opt()],
                )

                # ---- V path second: project the V shard (jjt-pair passes).
                # No bounce here: the V all-gather's DRAM traffic would
                # starve the K^T chunk prefetch right after the K gather, so
                # the bounce DMA is issued on the sync queue mid-prefetch
                # (see the attention phase below). ----
                vt = transpose_input(v_stg, "xt")
                for g in range(2):
                    jjts = range(2 * g, 2 * g + 2)
                    pss = {
                        (jjt, eh): ppsum.tile([P, EH], f32, tag="pps", name="pps")
                        for jjt in jjts for eh in range(n_eh)
                    }
                    for ct in range(n_ct):
                        for jjt in jjts:
                            jsl = slice(jjt * P, (jjt + 1) * P)
                            for eh in range(n_eh):
                                esl = slice(eh * EH, (eh + 1) * EH)
                                nc.tensor.matmul(
                                    pss[(jjt, eh)][:], vt[:, ct, jsl],
                                    wv[:, ct, esl],
                                    start=(ct == 0), stop=(ct == n_ct - 1),
                                )
                    for i, (jjt, eh) in enumerate(pss):
                        esl = slice(eh * EH, (eh + 1) * EH)
                        copy_eng(i)(v_loc[:, jjt, esl], pss[(jjt, eh)][:])

                # ---- Q path last (local only; needed first at S start) ----
                qt = transpose_input(q_stg, "xt")

                def q_out(i, dtt, ps):
                    copy_eng(i)(qthi[:, dtt], ps[:])

                project_dt(wq, qt, q_out)

            # ================= attention phase =================
            m_t = [statp.tile([P, 1], f32, tag=f"m{it}", name=f"m{it}") for it in range(n_it)]
            tmpmax = statp.tile([P, 1], f32, tag="tmpmax", name="tmpmax")
            bias_t = [statp.tile([P, 1], f32, tag=f"b{it}", name=f"b{it}") for it in range(n_it)]
            ell_t = [statp.tile([P, 1], f32, tag=f"l{it}", name=f"l{it}") for it in range(n_it)]
            rl_t = [statp.tile([P, 1], f32, tag=f"r{it}", name=f"r{it}") for it in range(n_it)]

            gk = gath_k.rearrange("(r dtt p) jj -> r p dtt jj", r=cores, p=P)
            gv = gath_v.rearrange("(jg jj p) e -> jg p jj e", jj=JG, p=P)

            with (
                tc.tile_pool(name="kall", bufs=1) as kallp,
                tc.tile_pool(name="srow", bufs=2) as srow,
                tc.tile_pool(name="prow", bufs=2) as prow,
                tc.tile_pool(name="ptp", bufs=1) as ptp,
                tc.tile_pool(name="vchunk", bufs=2) as vchunk,
                tc.tile_pool(name="opool", bufs=2) as opool,
            ):
                # ---- K^T prefetch: one 8MB tile, 8 chunk-major 1MB DMAs
                # (the order S consumes it: S row tile 0 streams chunk by
                # chunk). The V bounce + gather are slotted in after chunk 2
                # so the V transfer overlaps the prefetch tail and the whole
                # S phase, finishing just before PV needs it — while the
                # prefetch head (which paces S row tile 0) stays
                # collective-free. ----
                kall = kallp.tile([P, n_dt, nq], f16, tag="kall", name="kall")
                half = nq // 2
                bv = bounce_v.rearrange("(jjt p) e -> p jjt e", p=P)
                for rr in range(cores):
                    rsl = slice(rr * sh, (rr + 1) * sh)
                    nc.sync.dma_start(kall[:, :, rsl], gk[rr])


                # s_sb holds RAW scores (std ~2.6e5 — far beyond fp16 max, so
                # f32). Only 2 bufs: s_sb[it] is dead once exp(it) has run.
                s_sb = [srow.tile([P, nq], f32, tag="s", name="s") for _ in range(n_it)]
                p_sb = [prow.tile([P, nq], f16, tag="p", name="p") for _ in range(n_it)]
                pt = [
                    ptp.tile([P, n_jt, P], f16, tag=f"pt{it}", name=f"pt{it}")
                    for it in range(n_it)
                ]

                # ---- S = Q K^T, it-outer so softmax + P^T of row tile it
                # overlap S of it+1. Row tile 0 is rr-middle/dtt-inner so it
                # consumes the gathered chunks at 1MB granularity as the
                # prefetch streams them in; row tiles 1-3 (chunks resident)
                # run dtt-middle over 4-chunk groups so one stationary Q^T
                # load feeds 4 matmuls (explicit LDWEIGHTS).
                def s_stats(it, rr, ps):
                    if rr == 0:
                        nc.vector.reduce_max(
                            m_t[it][:], ps[:], axis=mybir.AxisListType.X
                        )
                    else:
                        nc.vector.reduce_max(
                            tmpmax[:], ps[:], axis=mybir.AxisListType.X
                        )
                        nc.vector.tensor_max(m_t[it][:], m_t[it][:], tmpmax[:])
                    # all S copies on the vector engine: the scalar queue
                    # carries exp + P^T + the V bounce, and an S copy stuck
                    # behind those would stall the PSUM pool rotation
                    nc.vector.tensor_copy(s_sb[it][:, rr * sh:(rr + 1) * sh], ps[:])

                _spsum_cm = tc.tile_pool(name="spsum", bufs=8, space="PSUM")
                spsum = _spsum_cm.__enter__()
                for it in range(n_it):
                    isl = slice(it * P, (it + 1) * P)
                    if it == 0:
                        for rr in range(cores):
                            rsl = slice(rr * sh, (rr + 1) * sh)
                            ps = spsum.tile([P, sh], f32, tag="sps", name="sps")
                            for dtt in range(n_dt):
                                nc.tensor.matmul(
                                    ps[:], qthi[:, dtt, isl], kall[:, dtt, rsl],
                                    start=(dtt == 0), stop=(dtt == n_dt - 1),
                                )
                            s_stats(it, rr, ps)
                    else:
                        for g in range(2):
                            chunks = range(4 * g, 4 * g + 4)
                            pss = {
                                c: spsum.tile([P, sh], f32, tag="sps", name="sps")
                                for c in chunks
                            }
                            for dtt in range(n_dt):
                                mm = group_ldw(qthi[:, dtt, isl])
                                for c in chunks:
                                    csl = slice(c * sh, (c + 1) * sh)
                                    mm(
                                        pss[c][:], qthi[:, dtt, isl],
                                        kall[:, dtt, csl],
                                        start=(dtt == 0), stop=(dtt == n_dt - 1),
                                    )
                            for c in chunks:
                                s_stats(it, c, pss[c])
                    # softmax for this row tile; P^T in two halves (on the
                    # sync queue, idle once the prefetch drains) so PV can
                    # start after the first halves land
                    nc.vector.tensor_scalar_mul(bias_t[it][:], m_t[it][:], -scale)
                    nc.scalar.activation(
                        p_sb[it][:], s_sb[it][:],
                        mybir.ActivationFunctionType.Exp,
                        bias=bias_t[it][:], scale=scale,
                        accum_out=ell_t[it][:],
                    )
                    nc.vector.reciprocal(rl_t[it][:], ell_t[it][:])
                    if it == 0:
                        # The V bounce rides the scalar queue behind exp(it0)
                        # — i.e. it executes only once row tile 0 has
                        # consumed the whole K^T prefetch. An active
                        # collective and local shared-DRAM reads starve each
                        # other, so the V transfer must wait for the
                        # prefetch to drain; it then overlaps S row tiles
                        # 1-3 and completes before PV needs V.
                        nc.scalar.dma_start(bv[:], v_loc[:])
                        nc.gpsimd.collective_compute(
                            "AllGather", mybir.AluOpType.bypass,
                            replica_groups=rg,
                            ins=[bounce_v.ap().opt()],
                            outs=[gath_v.ap().opt()],
                        )
                    nc.scalar.dma_start_transpose(
                        pt[it][:, : n_jt // 2], p_sb[it][:, :half]
                    )
                    nc.scalar.dma_start_transpose(
                        pt[it][:, n_jt // 2:], p_sb[it][:, half:]
                    )
                _spsum_cm.__exit__(None, None, None)

                # ---- O = (P @ V) / ell, all 8 PSUM banks, single V pass.
                # eh pairs share the stationary P^T tile (LDWEIGHTS elided).
                _pvpsum_cm = tc.tile_pool(name="pvpsum", bufs=n_it * n_eh, space="PSUM")
                pvpsum = _pvpsum_cm.__enter__()
                pso = {
                    (it, eh): pvpsum.tile([P, EH], f32, tag="pvps", name="pvps")
                    for it in range(n_it) for eh in range(n_eh)
                }
                for jg in range(n_jg):
                    vc = vchunk.tile([P, JG, dim], f16, tag="vc", name="vc")
                    for jj in range(JG):
                        nc.sync.dma_start(vc[:, jj], gv[jg][:, jj])
                    last = jg == n_jg - 1
                    for it in range(n_it):
                        for jj in range(JG):
                            mm = group_ldw(pt[it][:, jg * JG + jj])
                            for eh in range(n_eh):
                                esl = slice(eh * EH, (eh + 1) * EH)
                                mm(
                                    pso[(it, eh)][:],
                                    pt[it][:, jg * JG + jj],
                                    vc[:, jj, esl],
                                    start=(jg == 0 and jj == 0),
                                    stop=(last and jj == JG - 1),
                                )
                        if last:
                            # scale + store this row tile while the PE is
                            # still accumulating the remaining row tiles
                            o_sb = opool.tile([P, dim], f32, tag="o", name="o")
                            for eh in range(n_eh):
                                esl = slice(eh * EH, (eh + 1) * EH)
                                nc.vector.tensor_scalar_mul(
                                    o_sb[:, esl], pso[(it, eh)][:], rl_t[it][:]
                                )
                            nc.scalar.dma_start(
                                out_ext[it * P:(it + 1) * P, :], o_sb[:]
                            )
                _pvpsum_cm.__exit__(None, None, None)

    return nc


_CACHE = {}
RUN_KW = {}


def _get_nc():
    if "nc" not in _CACHE:
        _CACHE["nc"] = build_attention()
    return _CACHE["nc"]


def kernel(**inputs):
    from concourse.bass_utils import run_bass_kernel_spmd

    q = np.ascontiguousarray(np.asarray(inputs["q"], dtype=np.float32))
    k = np.ascontiguousarray(np.asarray(inputs["k"], dtype=np.float32))
    v = np.ascontiguousarray(np.asarray(inputs["v"], dtype=np.float32))
    W_q = np.ascontiguousarray(np.asarray(inputs["W_q"], dtype=np.float32))
    W_k = np.ascontiguousarray(np.asarray(inputs["W_k"], dtype=np.float32))
    W_v = np.ascontiguousarray(np.asarray(inputs["W_v"], dtype=np.float32))

    sh = N_Q // CORES
    in_maps = []
    for r in range(CORES):
        sl = slice(r * sh, (r + 1) * sh)
        in_maps.append({
            "q": q[sl], "k": k[sl], "v": v[sl],
            "W_q": W_q, "W_k": W_k, "W_v": W_v,
        })

    nc = _get_nc()
    if not nc.is_finalized():
        nc.finalize()
    res = run_bass_kernel_spmd(nc, in_maps, core_ids=list(range(CORES)), **RUN_KW)
    _CACHE["last_result"] = res
    out = np.concatenate([res.results[r]["out"] for r in range(CORES)], axis=0)
    return out


if __name__ == "__main__":
    import reference

    inputs = {kk: np.asarray(vv) for kk, vv in reference.setup_inputs().items()}
    out = kernel(**inputs)
    print("out shape:", out.shape, out.dtype)


# revision 42
# speedup vs baseline: 1.3165x; 1.1563x over previous
"""Distributed attention layer kernel for 8 TRN2 NeuronCores.

Reference computation (f32):
    Q = q @ W_q; K = k @ W_k; V = v @ W_v
    out = softmax((Q @ K^T)/sqrt(d_k)) @ V

Sharding: rows of q/k/v are split 8 ways (sequence parallel). Each core
projects its own shards, the K^T/V projections are all-gathered (fp16),
and each core computes its 512-row slice of the attention output.

Precision: projections run in f32r (fp32 operands, PE rounds mantissas
to 11 bits, full rate for free-dim >= 256) with f32 PSUM accumulation.
K^T/Q^T/V are downcast to fp16 for the attention matmuls (QK^T and PV
single plain fp16 matmuls, f32 accumulation). Softmax is f32 (ACT exp
with per-row max bias, fused row-sum). Measured end-to-end error vs the
f32 reference: ~8e-3 (gate 2e-2).
"""

import os
import sys

for _p in ("/opt/pypackages", "/opt/trn_rl_repo"):
    if _p not in sys.path:
        sys.path.insert(0, _p)

import numpy as np

N_Q, N_KV, DIM = 4096, 4096, 1024  # D_K = D_V = DIM (square weights)
CORES = 8

P = 128


def build_attention(nq=N_Q, dim=DIM, cores=CORES):
    """Build the per-core Bass graph (SPMD; identical on all cores)."""
    import concourse.bass as bass
    import concourse.mybir as mybir
    from concourse import bacc
    from concourse.masks import make_identity
    from concourse.tile import TileContext

    dt = mybir.dt
    f32, f32r, f16 = dt.float32, dt.float32r, dt.float16

    sh = nq // cores          # rows per core (512)
    n_ct = dim // P           # contraction tiles for projections (8)
    n_dt = dim // P           # d tiles (8)
    n_it = sh // P            # query-row tiles per core (4)
    n_jjt = sh // P           # kv-row tiles per core (4)
    n_jt = nq // P            # total kv j tiles (32)
    JG = 4                    # j-tiles per PV V-chunk
    n_jg = n_jt // JG         # V chunk count (8)
    EH = 512
    n_eh = dim // EH          # 512-wide output column halves (2)
    hd = dim // 2
    nh = n_dt // 2
    scale = 1.0 / float(np.sqrt(dim))

    nc = bacc.Bacc(num_devices=cores)

    # --- external I/O (per core: row shards of q/k/v, full weights) ---
    q_ext = nc.declare_dram_parameter("q", [sh, dim], f32, isOutput=False)
    k_ext = nc.declare_dram_parameter("k", [sh, dim], f32, isOutput=False)
    v_ext = nc.declare_dram_parameter("v", [sh, dim], f32, isOutput=False)
    wq_ext = nc.declare_dram_parameter("W_q", [dim, dim], f32r, isOutput=False)
    wk_ext = nc.declare_dram_parameter("W_k", [dim, dim], f32r, isOutput=False)
    wv_ext = nc.declare_dram_parameter("W_v", [dim, dim], f32r, isOutput=False)
    out_ext = nc.declare_dram_parameter("out", [sh, dim], f32, isOutput=True)

    # --- internal DRAM for collectives ---
    bounce_k = nc.dram_tensor("bounce_k", [dim, sh], f16)
    bounce_v = nc.dram_tensor("bounce_v", [sh, dim], f16)
    gath_k = nc.dram_tensor("gath_k", [cores * dim, sh], f16, addr_space="Shared")
    gath_v = nc.dram_tensor("gath_v", [cores * sh, dim], f16, addr_space="Shared")

    rg = [list(range(cores))]

    with TileContext(nc) as tc:
        with (
            tc.tile_pool(name="const", bufs=1) as constp,
            tc.tile_pool(name="qt", bufs=1) as qtp,
            tc.tile_pool(name="stats", bufs=1) as statp,
        ):
            # NOTE: make_identity/PE-transpose on float32r crashes walrus
            # codegen; transposes run in plain f32 and the psum result is
            # copy-cast (bit-identical) into float32r SBUF tiles.
            ident_f = constp.tile([P, P], f32, tag="idf", name="idf")
            make_identity(nc, ident_f)

            qthi = qtp.tile([P, n_dt, sh], f16, tag="qthi", name="qthi")
            # v_loc outlives the projection pools: its bounce DMA is issued
            # mid-S-phase to delay the V all-gather until the khi chunk
            # loads have drained (avoids DRAM contention with the gather)
            v_loc = qtp.tile([P, sh // P, dim], f16, tag="v_loc", name="v_loc")

            with (
                tc.tile_pool(name="w", bufs=1) as wpool,
                tc.tile_pool(name="iost", bufs=6) as iost,
                tc.tile_pool(name="tin", bufs=2) as tpool,
                tc.tile_pool(name="kvout", bufs=1) as kvout,
                tc.tile_pool(name="tpsum", bufs=4, space="PSUM") as tpsum,
                tc.tile_pool(name="ppsum", bufs=4, space="PSUM") as ppsum,
            ):
                # All bulk loads (inputs + weights) stream in order on the
                # sync (SP) HWDGE queue; the Activation HWDGE queue is kept
                # for small latency-critical transfers (bounce buffers, P^T
                # XBAR transposes, outputs) so their triggers never stall the
                # ACT engine behind megabytes of weight traffic.
                def load_input(x_ext):
                    stgs = []
                    xsrc = x_ext.rearrange("(it p) c -> p it c", p=P)
                    for it in range(sh // P):
                        stg = iost.tile([P, dim], f32, tag="iostg", name="iostg")
                        nc.sync.dma_start(stg[:], xsrc[:, it])
                        stgs.append(stg)
                    return stgs

                wk = wpool.tile([P, n_ct, dim], f32r, tag="wk", name="wk")
                wq = wpool.tile([P, n_ct, dim], f32r, tag="wq", name="wq")
                wv = wpool.tile([P, n_ct, dim], f32r, tag="wv", name="wv")
                wk_src = wk_ext.rearrange("(ct p) d -> p ct d", p=P)
                wq_src = wq_ext.rearrange("(ct p) d -> p ct d", p=P)
                wv_src = wv_ext.rearrange("(ct p) d -> p ct d", p=P)

                # interleave k row tiles with wk column tiles so the
                # ct-outer K projection's weights arrive right behind the
                # transposes instead of after the whole k shard
                k_stg = []
                ksrc = k_ext.rearrange("(it p) c -> p it c", p=P)
                for it in range(sh // P):
                    stg = iost.tile([P, dim], f32, tag="iostg", name="iostg")
                    nc.sync.dma_start(stg[:], ksrc[:, it])
                    k_stg.append(stg)
                    nc.sync.dma_start(wk[:, it], wk_src[:, it])
                for ct in range(sh // P, n_ct):
                    nc.sync.dma_start(wk[:, ct], wk_src[:, ct])
                q_stg = load_input(q_ext)
                nc.sync.dma_start(wq[:, :, :hd], wq_src[:, :, :hd])
                nc.sync.dma_start(wq[:, :, hd:], wq_src[:, :, hd:])
                nc.sync.dma_start(wv[:, :, :hd], wv_src[:, :, :hd])
                nc.sync.dma_start(wv[:, :, hd:], wv_src[:, :, hd:])

                def transpose_input(stgs, tag):
                    """Transpose a staged [sh, dim] f32 input on the PE into a
                    [c_in=128, ct, row] f32r SBUF tile (copy-cast from psum)."""
                    xt = tpool.tile([P, n_ct, sh], f32r, tag=tag, name=tag)
                    for it, stg in enumerate(stgs):
                        dst = slice(it * P, (it + 1) * P)
                        for ct in range(n_ct):
                            ps = tpsum.tile([P, P], f32, tag="tps", name="tps")
                            nc.tensor.transpose(
                                ps[:], stg[:, ct * P:(ct + 1) * P], ident_f
                            )
                            nc.vector.tensor_copy(xt[:, ct, dst], ps[:])
                    return xt

                # ---- K path first: project K^T, bounce out, all-gather.
                # Single gather: the kernel-entry CC barrier (~45-55us of
                # launch skew) gates the first collective anyway, and Shared
                # DRAM reads starve while any collective is active, so one
                # gather followed by a full-speed khi prefetch beats split
                # gathers whose chunk reads crawl under the second one. ----
                kt = transpose_input(k_stg, "xt")
                kt_loc = kvout.tile([P, n_dt, sh], f16, tag="kt_loc", name="kt_loc")
                bk = bounce_k.rearrange("(dtt p) jj -> p dtt jj", p=P)
                # ct-outer two-pass projection: pass g's 4 PSUM banks start
                # as soon as wk[ct=0] lands (interleaved with the k tiles
                # above) instead of waiting for the full 4MB weight load,
                # and each pass bounces its half of K^T immediately — the
                # all-gather triggers ~7us earlier, which moves the gather
                # end (gated by the LAST core's trigger via launch skew).
                for g in range(2):
                    dts = range(4 * g, 4 * g + 4)
                    pss = {
                        dtt: ppsum.tile([P, sh], f32, tag="pps", name="pps")
                        for dtt in dts
                    }
                    for ct in range(n_ct):
                        for dtt in dts:
                            dsl = slice(dtt * P, (dtt + 1) * P)
                            nc.tensor.matmul(
                                pss[dtt][:], wk[:, ct, dsl], kt[:, ct],
                                start=(ct == 0), stop=(ct == n_ct - 1),
                            )
                    for dtt in dts:
                        nc.scalar.copy(kt_loc[:, dtt], pss[dtt][:])
                    hs = slice(4 * g, 4 * g + 4)
                    nc.scalar.dma_start(bk[:, hs], kt_loc[:, hs])
                nc.gpsimd.collective_compute(
                    "AllGather", mybir.AluOpType.bypass, replica_groups=rg,
                    ins=[bounce_k.ap().opt()], outs=[gath_k.ap().opt()],
                )

                # ---- Q path (local only): project Q^T, downcast to fp16 ----
                qt = transpose_input(q_stg, "xt")
                for dtt in range(n_dt):
                    ps = ppsum.tile([P, sh], f32, tag="pps", name="pps")
                    dsl = slice(dtt * P, (dtt + 1) * P)
                    for ct in range(n_ct):
                        nc.tensor.matmul(
                            ps[:], wq[:, ct, dsl], qt[:, ct],
                            start=(ct == 0), stop=(ct == n_ct - 1),
                        )
                    nc.scalar.copy(qthi[:, dtt], ps[:])

                # ---- V path: project V shard, downcast ----
                v_stg = load_input(v_ext)
                vt = transpose_input(v_stg, "xt")
                for jjt in range(n_jjt):
                    jsl = slice(jjt * P, (jjt + 1) * P)
                    for eh in range(n_eh):
                        ps = ppsum.tile([P, EH], f32, tag="pps", name="pps")
                        esl = slice(eh * EH, (eh + 1) * EH)
                        for ct in range(n_ct):
                            nc.tensor.matmul(
                                ps[:], vt[:, ct, jsl], wv[:, ct, esl],
                                start=(ct == 0), stop=(ct == n_ct - 1),
                            )
                        nc.scalar.copy(v_loc[:, jjt, esl], ps[:])

            # ================= attention phase =================
            m_t = [statp.tile([P, 1], f32, tag=f"m{it}", name=f"m{it}") for it in range(n_it)]
            tmpmax = statp.tile([P, 1], f32, tag="tmpmax", name="tmpmax")
            bias_t = [statp.tile([P, 1], f32, tag=f"b{it}", name=f"b{it}") for it in range(n_it)]
            ell_t = [statp.tile([P, 1], f32, tag=f"l{it}", name=f"l{it}") for it in range(n_it)]
            rl_t = [statp.tile([P, 1], f32, tag=f"r{it}", name=f"r{it}") for it in range(n_it)]

            gk = gath_k.rearrange("(r dtt p) jj -> r p dtt jj", r=cores, p=P)
            gv = gath_v.rearrange("(jg jj p) e -> jg p jj e", jj=JG, p=P)
            bv = bounce_v.rearrange("(jjt p) e -> p jjt e", p=P)

            with (
                tc.tile_pool(name="schunk", bufs=5) as schunk,
                tc.tile_pool(name="srow", bufs=n_it) as srow,
                tc.tile_pool(name="prow", bufs=2) as prow,
                tc.tile_pool(name="ptp", bufs=1) as ptp,
                tc.tile_pool(name="vchunk", bufs=3) as vchunk,
                tc.tile_pool(name="opool", bufs=2) as opool,
            ):
                s_sb = [srow.tile([P, nq], f32, tag="s", name="s") for _ in range(n_it)]

                # ---- scores: all khi chunk loads issued upfront (they
                # stream at full bandwidth in the collective-free window
                # right after the K gather), then S with running row max.
                # The V gather is released only after rr==2 so it does not
                # starve the tail of the khi prefetch. ----
                khis = []
                for rr in range(cores):
                    khi = schunk.tile([P, n_dt, sh], f16, tag="khi", name="khi")
                    # two half-loads per chunk: the first 4 dtt matmuls start
                    # as soon as half the bytes land
                    nc.sync.dma_start(khi[:, :nh], gk[rr][:, :nh])
                    nc.sync.dma_start(khi[:, nh:], gk[rr][:, nh:])
                    khis.append(khi)

                p_sb = [prow.tile([P, nq], f16, tag="p", name="p") for _ in range(n_it)]
                pt = [
                    ptp.tile([P, n_jt, P], f16, tag=f"pt{it}", name=f"pt{it}")
                    for it in range(n_it)
                ]

                _spsum_cm = tc.tile_pool(name="spsum", bufs=6, space="PSUM")
                spsum = _spsum_cm.__enter__()
                for rr in range(cores):
                    rsl = slice(rr * sh, (rr + 1) * sh)
                    for it in range(n_it):
                        isl = slice(it * P, (it + 1) * P)
                        ps = spsum.tile([P, sh], f32, tag="sps", name="sps")
                        for dtt in range(n_dt):
                            nc.tensor.matmul(
                                ps[:], qthi[:, dtt, isl], khis[rr][:, dtt],
                                start=(dtt == 0), stop=(dtt == n_dt - 1),
                            )
                        if rr == 0:
                            nc.vector.reduce_max(
                                m_t[it][:], ps[:], axis=mybir.AxisListType.X
                            )
                        else:
                            nc.vector.reduce_max(
                                tmpmax[:], ps[:], axis=mybir.AxisListType.X
                            )
                            nc.vector.tensor_max(m_t[it][:], m_t[it][:], tmpmax[:])
                        if rr < cores - 1:
                            nc.scalar.copy(s_sb[it][:, rsl], ps[:])
                        else:
                            # last chunk's copies go on the vector engine so
                            # the inline exps below don't delay them (they
                            # gate the S->PV PSUM pool handover)
                            nc.vector.tensor_copy(s_sb[it][:, rsl], ps[:])
                        if rr == cores - 1:
                            # softmax fires per row tile as soon as its last
                            # chunk lands: exp(it) on ACT and the P^T XBAR
                            # transpose overlap the remaining S matmuls
                            nc.vector.tensor_scalar_mul(
                                bias_t[it][:], m_t[it][:], -scale
                            )
                            nc.scalar.activation(
                                p_sb[it][:], s_sb[it][:],
                                mybir.ActivationFunctionType.Exp,
                                bias=bias_t[it][:], scale=scale,
                                accum_out=ell_t[it][:],
                            )
                            nc.vector.reciprocal(rl_t[it][:], ell_t[it][:])
                            nc.scalar.dma_start_transpose(pt[it][:], p_sb[it][:])
                    if rr == 2:
                        # bounce rides the scalar queue behind rr<=2's copies,
                        # so the V gather starts only once the khi prefetch
                        # has drained; gpsimd emission stays after the K
                        # collective so khi loads never wait on its tick.
                        nc.scalar.dma_start(bv[:], v_loc[:])
                        nc.gpsimd.collective_compute(
                            "AllGather", mybir.AluOpType.bypass, replica_groups=rg,
                            ins=[bounce_v.ap().opt()], outs=[gath_v.ap().opt()],
                        )
                _spsum_cm.__exit__(None, None, None)

                # ---- O = (P @ V) / ell, all 8 PSUM banks, single V pass ----
                _pvpsum_cm = tc.tile_pool(name="pvpsum", bufs=n_it * n_eh, space="PSUM")
                pvpsum = _pvpsum_cm.__enter__()
                pso = {
                    (it, eh): pvpsum.tile([P, EH], f32, tag="pvps", name="pvps")
                    for it in range(n_it) for eh in range(n_eh)
                }
                for jg in range(n_jg):
                    vc = vchunk.tile([P, JG, dim], f16, tag="vc", name="vc")
                    # per-j-tile loads: matmuls on jj consume each quarter as
                    # it lands instead of waiting for the full 2 MB chunk
                    for jj in range(JG):
                        nc.sync.dma_start(vc[:, jj], gv[jg][:, jj])
                    last = jg == n_jg - 1
                    for it in range(n_it):
                        for eh in range(n_eh):
                            esl = slice(eh * EH, (eh + 1) * EH)
                            for jj in range(JG):
                                nc.tensor.matmul(
                                    pso[(it, eh)][:],
                                    pt[it][:, jg * JG + jj],
                                    vc[:, jj, esl],
                                    start=(jg == 0 and jj == 0),
                                    stop=(last and jj == JG - 1),
                                )
                        if last:
                            # scale + store this row tile while the PE is
                            # still accumulating the remaining row tiles
                            o_sb = opool.tile([P, dim], f32, tag="o", name="o")
                            for eh in range(n_eh):
                                esl = slice(eh * EH, (eh + 1) * EH)
                                nc.vector.tensor_scalar_mul(
                                    o_sb[:, esl], pso[(it, eh)][:], rl_t[it][:]
                                )
                            nc.scalar.dma_start(
                                out_ext[it * P:(it + 1) * P, :], o_sb[:]
                            )
                _pvpsum_cm.__exit__(None, None, None)

    return nc


_CACHE = {}
RUN_KW = {}


def _get_nc():
    if "nc" not in _CACHE:
        _CACHE["nc"] = build_attention()
    return _CACHE["nc"]


def kernel(**inputs):
    from concourse.bass_utils import run_bass_kernel_spmd

    q = np.ascontiguousarray(np.asarray(inputs["q"], dtype=np.float32))
    k = np.ascontiguousarray(np.asarray(inputs["k"], dtype=np.float32))
    v = np.ascontiguousarray(np.asarray(inputs["v"], dtype=np.float32))
    W_q = np.ascontiguousarray(np.asarray(inputs["W_q"], dtype=np.float32))
    W_k = np.ascontiguousarray(np.asarray(inputs["W_k"], dtype=np.float32))
    W_v = np.ascontiguousarray(np.asarray(inputs["W_v"], dtype=np.float32))

    sh = N_Q // CORES
    in_maps = []
    for r in range(CORES):
        sl = slice(r * sh, (r + 1) * sh)
        in_maps.append({
            "q": q[sl], "k": k[sl], "v": v[sl],
            "W_q": W_q, "W_k": W_k, "W_v": W_v,
        })

    nc = _get_nc()
    if not nc.is_finalized():
        nc.finalize()
    res = run_bass_kernel_spmd(nc, in_maps, core_ids=list(range(CORES)), **RUN_KW)
    _CACHE["last_result"] = res
    out = np.concatenate([res.results[r]["out"] for r in range(CORES)], axis=0)
    return out


if __name__ == "__main__":
    import reference

    inputs = {kk: np.asarray(vv) for kk, vv in reference.setup_inputs().items()}
    out = kernel(**inputs)
    print("out shape:", out.shape, out.dtype)

